# revision 1
# baseline (speedup 1.0000x reference)
"""Segment-sum (AggrSum) kernel for 8 Trainium2 NeuronCores.

Math: out[v, :] = sum_{n: X_neis[n] == v} H[n, :]   (H [N, D], out [V, D])

Strategy (V-sharding with host-side bucketing as the sharding step):
  - Sort edge ids by target vocab index; group edges by 128-row vocab tile.
  - Partition the 64 vocab tiles into 8 balanced groups of 8 (one per
    core), ordered inside each group so that packed prefix drift stays
    within +-128 rows of 512*vt ("mode B"). Each core then reads an
    exactly-packed, zero-padding-free edge stream; every vocab tile's
    edges are covered by a fixed window of 6 physical 128-row tiles at
    static offsets, and the one-hot masks zero out foreign rows.
  - H rows are uploaded as an exact fp16 hi/lo split (hi = fp16(H),
    lo = fp16(H - hi); the sum reproduces fp32 to ~1e-7); each (vt, k)
    window tile needs two fp16 matmuls (hi, lo) accumulating into the
    same [128, 256] fp32 PSUM tile. One DVE is_equal per vocab tile
    builds all six masks at once. Each core writes its own eight output
    tiles; the host scatters them into the full [V, D].
  - If the drift bound fails (pathological inputs), fall back to
    "mode A": pad every vocab tile to K*128 rows (no windows).
"""

import numpy as np

import concourse.bacc as bacc
import concourse.mybir as mybir
import concourse.tile as tile
from concourse.bass_utils import run_bass_kernel_spmd

N, D, V = 32768, 256, 8192
NCORES = 8
P = 128
VT_PER_CORE = V // P // NCORES  # 8 vocab tiles of 128 per core
NVT = V // P  # 64 global vocab tiles
KB = 6  # window tiles per vocab tile in mode B
NTILES_B = 4 * (VT_PER_CORE - 1) - 1 + KB  # 33 physical tiles per core
W = 2 * D  # hi|lo concatenated row width

TRACE = False
LAST_EXEC_NS = None
LAST_RESULTS = None

_PROGRAM_CACHE: dict = {}


def _win_lo(vt: int) -> int:
    """First physical tile of vocab tile vt's window (mode B)."""
    return 0 if vt == 0 else 4 * vt - 1


def _build_common(nc, tc, pools, K, n_phys_tiles, hs, consts, out, chunk_tiles):
    f32 = mybir.dt.float32
    f16 = mybir.dt.float16
    const_pool, hpool, mpool, opool, psum_pool = pools
    nconst = VT_PER_CORE * K + P
    iota_off = VT_PER_CORE * K

    # Warm up the PE's HAM clock (1.2 -> 2.4 GHz needs ~4us of sustained
    # matmul activity). Dummies run on a memset scratch tile with no DMA
    # dependency, so they start right after the engine preamble while the
    # real operands are still streaming in; without this the first ~3us of
    # real matmuls run at half clock and the PE trails the input stream.
    warm_sb = const_pool.tile([P, P], f16, name="warm_sb", tag="warmsb")
    nc.gpsimd.memset(warm_sb[:], 0.0)
    warm_ps = psum_pool.tile([P, 64], f32, name="warm", tag="warm", bufs=1)
    for _ in range(72):
        nc.tensor.matmul(
            out=warm_ps[:],
            lhsT=warm_sb[:],
            rhs=warm_sb[:, :64],
            start=True,
            stop=True,
        )

    const_sb = const_pool.tile([P, nconst], f16)
    nc.sync.dma_start(const_sb[:], consts[:])

    # chunked prefetch of the packed H stream
    chunks = []
    t0 = 0
    for ct in chunk_tiles:
        ch = hpool.tile([P, ct * W], f16, name="ch")
        nc.sync.dma_start(ch[:], hs[:, t0 * W : (t0 + ct) * W])
        chunks.append((t0, ct, ch))
        t0 += ct
    assert t0 == n_phys_tiles

    def rhs_slice(t, plane):
        for t0, ct, ch in chunks:
            if t0 <= t < t0 + ct:
                off = (t - t0) * W + plane * D
                return ch[:, off : off + D]
        raise AssertionError(t)

    # single mask and output buffers, slice-written: fewer pool tiles means
    # fewer TileRelease events in the kernel epilogue
    big_m = mpool.tile([P, VT_PER_CORE * K * P], f16, name="big_m", bufs=1)
    big_ot = opool.tile([P, VT_PER_CORE * D], f32, name="big_ot", bufs=1)

    def emit_mask(vt):
        # all K masks for vocab tile vt in one DVE op:
        # m[p, k, q] = (xrel[p, vt*K+k] == iota[q])
        m = big_m[:, vt * K * P : (vt + 1) * K * P]
        nc.vector.tensor_tensor(
            out=m.rearrange("p (k q) -> p k q", k=K),
            in0=const_sb[:, vt * K : (vt + 1) * K]
            .unsqueeze(2)
            .broadcast_to([P, K, P]),
            in1=const_sb[:, iota_off : iota_off + P]
            .unsqueeze(1)
            .broadcast_to([P, K, P]),
            op=mybir.AluOpType.is_equal,
        )
        return m

    # DVE stream order: a few masks ahead, then copies interleaved with the
    # remaining masks so output DMAs start streaming early
    AHEAD = 3
    ms = [emit_mask(vt) for vt in range(min(AHEAD, VT_PER_CORE))]

    for vt in range(VT_PER_CORE):
        m = ms[vt]
        # hi and lo planes accumulate into the same [P, D] psum
        ps = psum_pool.tile([P, D], f32, name="ps")
        for k in range(K):
            t = vt * K + k if n_phys_tiles == VT_PER_CORE * K else _win_lo(vt) + k
            for plane in range(2):
                nc.tensor.matmul(
                    out=ps[:],
                    lhsT=m[:, k * P : (k + 1) * P],
                    rhs=rhs_slice(t, plane),
                    start=(k == 0 and plane == 0),
                    stop=(k == K - 1 and plane == 1),
                )
        if vt + AHEAD < VT_PER_CORE:
            ms.append(emit_mask(vt + AHEAD))
        ot = big_ot[:, vt * D : (vt + 1) * D]
        nc.vector.tensor_copy(ot, ps[:])
        nc.scalar.dma_start(out[vt * P : (vt + 1) * P, :], ot)


def _build_program(mode, K):
    """mode 'B': exact-packed windows (K=KB); mode 'A': padded (K tiles/vt)."""
    f32 = mybir.dt.float32
    f16 = mybir.dt.float16
    if mode == "B":
        n_phys = NTILES_B
        chunk_tiles = [2, 4, 6, 7, 7, 4, 2, 1]
    else:
        n_phys = VT_PER_CORE * K
        nt = n_phys
        chunk_tiles = []
        while nt > 0:
            chunk_tiles.append(min(7, nt))
            nt -= min(7, nt)
    nconst = VT_PER_CORE * K + P

    nc = bacc.Bacc("TRN2", target_bir_lowering=False)
    hs = nc.dram_tensor("hs", [P, n_phys * W], f16, kind="ExternalInput")
    consts = nc.dram_tensor("consts", [P, nconst], f16, kind="ExternalInput")
    out = nc.dram_tensor("out", [VT_PER_CORE * P, D], f32, kind="ExternalOutput")

    with tile.TileContext(nc) as tc:
        with (
            tc.tile_pool(name="const", bufs=1) as const_pool,
            tc.tile_pool(name="h", bufs=min(len(chunk_tiles), 16)) as hpool,
            tc.tile_pool(name="m", bufs=VT_PER_CORE) as mpool,
            tc.tile_pool(name="o", bufs=4) as opool,
            tc.tile_pool(name="psum", bufs=VT_PER_CORE - 1, space="PSUM") as psum_pool,
        ):
            _build_common(
                nc,
                tc,
                (const_pool, hpool, mpool, opool, psum_pool),
                K,
                n_phys,
                hs,
                consts,
                out,
                chunk_tiles,
            )
    nc.finalize()
    return nc


def _partition_tiles(counts):
    """Partition the 64 vocab tiles into 8 groups of 8, ordered so packed
    prefix drift stays in [-128, 128]. Returns groups (list of lists of
    global tile ids) or None if the bound fails."""
    order = np.argsort(counts)[::-1]  # descending by count
    # snake-deal into 8 groups to balance totals
    groups = [[] for _ in range(NCORES)]
    for i, g in enumerate(order):
        rnd, pos = divmod(i, NCORES)
        c = pos if rnd % 2 == 0 else NCORES - 1 - pos
        groups[c].append(int(g))
    final = []
    for c in range(NCORES):
        tiles = sorted(groups[c], key=lambda g: -counts[g])
        # alternate large/small: c0, c7, c1, c6, ...
        seq = []
        i, j = 0, len(tiles) - 1
        while i <= j:
            seq.append(tiles[i])
            if i != j:
                seq.append(tiles[j])
            i += 1
            j -= 1
        # verify drift bound
        run = 0
        for k, g in enumerate(seq):
            drift = run - 512 * k
            if not (-128 <= drift <= 128):
                return None
            run += int(counts[g])
            if k == 0 and run > 768:
                return None
        if not (-128 <= run - 4096 <= 128):
            return None
        final.append(seq)
    return final


def _iota_np():
    return np.tile(np.arange(P, dtype=np.float32), (P, 1))


def _pack_consts(xr, iota_np):
    return np.hstack([xr, iota_np]).astype(np.float16)


def _split_f16(block):
    hi = block.astype(np.float16)
    lo = (block - hi.astype(np.float32)).astype(np.float16)
    return hi, lo


def _tilemajor(hi, lo, ntiles):
    """[ntiles*P, D] hi/lo -> [P, ntiles*W] with per-tile [hi|lo] rows."""
    return (
        np.stack([hi, lo], axis=1)
        .reshape(ntiles, P, W)
        .transpose(1, 0, 2)
        .reshape(P, ntiles * W)
    )


def _shard_mode_b(H, X, order, Xs, counts, starts, groups):
    in_maps = []
    scatter = []  # (core, vt) -> global tile id
    iota_np = _iota_np()
    for c in range(NCORES):
        seq = groups[c]
        rows = np.concatenate([order[starts[g] : starts[g + 1]] for g in seq])
        xval = np.concatenate(
            [Xs[starts[g] : starts[g + 1]] for g in seq]
        ).astype(np.float64)
        n_c = len(rows)
        block = np.zeros((NTILES_B * P, D), dtype=np.float32)
        block[:n_c] = H[rows]
        xpad = np.full(NTILES_B * P, -1000.0, dtype=np.float64)
        xpad[:n_c] = xval
        hi, lo = _split_f16(block)
        hs = _tilemajor(hi, lo, NTILES_B)
        xr = np.full((P, VT_PER_CORE * KB), -1000.0, dtype=np.float32)
        for vt in range(VT_PER_CORE):
            base = 128.0 * seq[vt]
            for k in range(KB):
                t = _win_lo(vt) + k
                xr[:, vt * KB + k] = (xpad[t * P : (t + 1) * P] - base).astype(
                    np.float32
                )
        in_maps.append({"hs": hs, "consts": _pack_consts(xr, iota_np)})
        scatter.append(seq)
    return in_maps, scatter


def _shard_mode_a(H, X, order, Xs, counts, starts, K):
    in_maps = []
    scatter = []
    iota_np = _iota_np()
    for c in range(NCORES):
        hs = np.zeros((P, VT_PER_CORE * K * W), dtype=np.float16)
        xr = np.full((P, VT_PER_CORE * K), -1000.0, dtype=np.float32)
        seq = list(range(c * VT_PER_CORE, (c + 1) * VT_PER_CORE))
        for vt, g in enumerate(seq):
            s, e = int(starts[g]), int(starts[g + 1])
            cnt = e - s
            block = np.zeros((K * P, D), dtype=np.float32)
            block[:cnt] = H[order[s:e]]
            hi, lo = _split_f16(block)
            hs[:, vt * K * W : (vt + 1) * K * W] = _tilemajor(hi, lo, K)
            xv = np.full(K * P, -1000.0, dtype=np.float32)
            xv[:cnt] = (Xs[s:e] - g * P).astype(np.float32)
            xr[:, vt * K : (vt + 1) * K] = xv.reshape(K, P).T
        in_maps.append({"hs": hs, "consts": _pack_consts(xr, iota_np)})
        scatter.append(seq)
    return in_maps, scatter


def kernel(H, X_neis, V=V):
    global LAST_EXEC_NS, LAST_RESULTS
    H = np.asarray(H, dtype=np.float32)
    X = np.asarray(X_neis).astype(np.int64)
    assert H.shape == (N, D) and X.shape == (N,)

    order = np.argsort(X, kind="stable")
    Xs = X[order]
    counts = np.bincount(X, minlength=V).reshape(NVT, P).sum(axis=1)
    starts = np.zeros(NVT + 1, dtype=np.int64)
    np.cumsum(counts, out=starts[1:])

    groups = _partition_tiles(counts)
    if groups is not None:
        mode, K = "B", KB
        in_maps, scatter = _shard_mode_b(H, X, order, Xs, counts, starts, groups)
    else:
        mode, K = "A", max(1, int(-(-counts.max() // P)))
        in_maps, scatter = _shard_mode_a(H, X, order, Xs, counts, starts, K)

    key = (mode, K)
    if key not in _PROGRAM_CACHE:
        _PROGRAM_CACHE[key] = _build_program(mode, K)
    nc = _PROGRAM_CACHE[key]

    try:
        res = run_bass_kernel_spmd(nc, in_maps, list(range(NCORES)), trace=TRACE)
    except Exception:
        # transient NRT/device hiccups have been observed; retry once
        res = run_bass_kernel_spmd(nc, in_maps, list(range(NCORES)), trace=TRACE)
    LAST_EXEC_NS = res.exec_time_ns
    LAST_RESULTS = res

    full = np.empty((V, D), dtype=np.float32)
    for c in range(NCORES):
        o = res.results[c]["out"]
        for vt, g in enumerate(scatter[c]):
            full[g * P : (g + 1) * P] = o[vt * P : (vt + 1) * P]
    return full



# revision 5
# speedup vs baseline: 1.3388x; 1.3388x over previous
"""Segment-sum (AggrSum) kernel for 8 Trainium2 NeuronCores.

Math: out[v, :] = sum_{n: X_neis[n] == v} H[n, :]   (H [N, D], out [V, D])

Strategy (V-sharding with host-side bucketing as the sharding step):
  - Sort edge ids by target vocab index; group edges by 128-row vocab tile.
  - Partition the 64 vocab tiles into 8 balanced groups of 8 (one per
    core), ordered inside each group so that packed prefix drift stays
    within [0, 128] rows of 512*vt ("mode B5"). Each core reads an
    exactly-packed edge stream; every vocab tile's edges are covered by
    a fixed window of K=5 physical 128-row tiles at static offsets, and
    the one-hot masks zero out foreign rows.
  - H rows are uploaded as a single fp16 plane (rel err ~3e-4, well
    under the 2e-2 gate); one fp16 matmul per (vt, k) window tile
    accumulates into a [128, 256] fp32 PSUM tile.  Mask emission
    (one DVE is_equal per vocab tile) is split across the Vector and
    GpSimd engines so neither gates the DMA-bound stream.  PSUM->SBUF
    copies run on the Scalar (ACT) engine with an fp32->fp16 convert;
    outputs stream back as fp16 and the host upconverts + scatters.
  - Fallbacks: drift in [-128,128] with K=6 windows ("mode B6"), then
    padded per-vt tiles ("mode A") for pathological inputs.
"""

import numpy as np

import concourse.bacc as bacc
import concourse.mybir as mybir
import concourse.tile as tile
from concourse.bass_utils import run_bass_kernel_spmd

N, D, V = 32768, 256, 8192
NCORES = 8
P = 128
VT_PER_CORE = V // P // NCORES  # 8 vocab tiles of 128 per core
NVT = V // P  # 64 global vocab tiles
NTILES_B = 33  # physical 128-row tiles per core in mode B (K=5 and K=6)
N_WARMUP = 16

TRACE = False
LAST_EXEC_NS = None
LAST_RESULTS = None

_PROGRAM_CACHE: dict = {}

# vt -> mask-emission engine: 'v' (Vector) or 'g' (GpSimd).  TRN2's Pool
# (gpsimd) slot rejects TENSOR_TENSOR at codegen, so everything stays on
# Vector until a Pool-legal expansion path is proven.
MASK_ENG = ["v", "v", "v", "v", "v", "v", "v", "v"]


def _win_lo(mode: str, K: int, vt: int) -> int:
    """First physical tile of vocab tile vt's window."""
    if mode == "B5":
        return 4 * vt
    if mode == "B6":
        return 0 if vt == 0 else 4 * vt - 1
    return vt * K  # mode A: padded, disjoint windows


def _build_common(nc, tc, pools, mode, K, n_phys_tiles, hs, consts, out, chunk_tiles):
    f32 = mybir.dt.float32
    f16 = mybir.dt.float16
    const_pool, hpool, mv_pool, mg_pool, opool, psum_pool, warm_pool = pools
    nconst = VT_PER_CORE * K + P
    iota_off = VT_PER_CORE * K

    # Warm up the PE's HAM clock gate (throttled 1.2 GHz until ~3.4us of
    # sustained matmul activity).  Dummies run on a memset scratch tile with
    # no DMA dependency, so they start right after the engine preamble while
    # the real operands are still streaming in.
    warm_sb = warm_pool.tile([P, P], f16, name="warm_sb", tag="warmsb")
    nc.gpsimd.memset(warm_sb[:], 0.0)
    warm_ps = psum_pool.tile([P, 64], f32, name="warm", tag="warm", bufs=1)
    for _ in range(N_WARMUP):
        nc.tensor.matmul(
            out=warm_ps[:],
            lhsT=warm_sb[:],
            rhs=warm_sb[:, :64],
            start=True,
            stop=True,
        )

    # Input streams: packed H chunks on Sync (HWDGE), consts on Scalar so
    # the first H bytes and the mask operands land concurrently.
    chunks = []
    t0 = 0
    chs = []
    for ct in chunk_tiles:
        chs.append(hpool.tile([P, ct * D], f16, name="ch"))
    const_sb = const_pool.tile([P, nconst], f16)
    nc.scalar.dma_start(const_sb[:], consts[:])
    for ct, ch in zip(chunk_tiles, chs):
        nc.sync.dma_start(ch[:], hs[:, t0 * D : (t0 + ct) * D])
        chunks.append((t0, ct, ch))
        t0 += ct
    assert t0 == n_phys_tiles

    def rhs_slice(t):
        for c0, ct, ch in chunks:
            if c0 <= t < c0 + ct:
                off = (t - c0) * D
                return ch[:, off : off + D]
        raise AssertionError(t)

    # Mask emission split across Vector and GpSimd; each engine owns one
    # slice-written buffer (subtile deps keep the matmuls fine-grained).
    n_v = MASK_ENG.count("v")
    n_g = MASK_ENG.count("g")
    big_mv = mv_pool.tile([P, max(n_v, 1) * K * P], f16, name="big_mv", bufs=1)
    big_mg = mg_pool.tile([P, max(n_g, 1) * K * P], f16, name="big_mg", bufs=1)
    slot = {"v": 0, "g": 0}
    mask_of = {}
    for vt in range(VT_PER_CORE):
        e = MASK_ENG[vt]
        s = slot[e]
        slot[e] += 1
        buf = big_mv if e == "v" else big_mg
        mask_of[vt] = buf[:, s * K * P : (s + 1) * K * P]

    def emit_mask(vt):
        # all K masks for vocab tile vt in one op:
        # m[p, k, q] = (xrel[p, vt*K+k] == iota[q])
        m = mask_of[vt]
        eng = nc.vector if MASK_ENG[vt] == "v" else nc.gpsimd
        eng.tensor_tensor(
            out=m.rearrange("p (k q) -> p k q", k=K),
            in0=const_sb[:, vt * K : (vt + 1) * K]
            .unsqueeze(2)
            .broadcast_to([P, K, P]),
            in1=const_sb[:, iota_off : iota_off + P]
            .unsqueeze(1)
            .broadcast_to([P, K, P]),
            op=mybir.AluOpType.is_equal,
        )
        return m

    ms = [emit_mask(vt) for vt in range(VT_PER_CORE)]

    big_ot = opool.tile([P, VT_PER_CORE * D], f16, name="big_ot", bufs=1)

    for vt in range(VT_PER_CORE):
        m = ms[vt]
        ps = psum_pool.tile([P, D], f32, name="ps")
        for k in range(K):
            t = _win_lo(mode, K, vt) + k
            nc.tensor.matmul(
                out=ps[:],
                lhsT=m[:, k * P : (k + 1) * P],
                rhs=rhs_slice(t),
                start=(k == 0),
                stop=(k == K - 1),
            )
        ot = big_ot[:, vt * D : (vt + 1) * D]
        nc.scalar.copy(ot, ps[:])
        # Sync is idle once the input chunk descriptors are out; keep the
        # Scalar queue free for the psum->sbuf copies.
        nc.sync.dma_start(out[vt * P : (vt + 1) * P, :], ot)


def _build_program(mode, K):
    """mode 'B5'/'B6': exact-packed windows; mode 'A': padded (K tiles/vt)."""
    f16 = mybir.dt.float16
    if mode in ("B5", "B6"):
        n_phys = NTILES_B
        chunk_tiles = [1, 6, 8, 9, 9]
    else:
        n_phys = VT_PER_CORE * K
        nt = n_phys
        chunk_tiles = []
        while nt > 0:
            chunk_tiles.append(min(7, nt))
            nt -= min(7, nt)
    nconst = VT_PER_CORE * K + P

    nc = bacc.Bacc("TRN2", target_bir_lowering=False)
    hs = nc.dram_tensor("hs", [P, n_phys * D], f16, kind="ExternalInput")
    consts = nc.dram_tensor("consts", [P, nconst], f16, kind="ExternalInput")
    out = nc.dram_tensor("out", [VT_PER_CORE * P, D], f16, kind="ExternalOutput")

    with tile.TileContext(nc) as tc:
        with (
            tc.tile_pool(name="const", bufs=1) as const_pool,
            tc.tile_pool(name="h", bufs=min(len(chunk_tiles), 16)) as hpool,
            tc.tile_pool(name="mv", bufs=1) as mv_pool,
            tc.tile_pool(name="mg", bufs=1) as mg_pool,
            tc.tile_pool(name="o", bufs=1) as opool,
            tc.tile_pool(name="warm", bufs=1) as warm_pool,
            tc.tile_pool(name="psum", bufs=4, space="PSUM") as psum_pool,
        ):
            _build_common(
                nc,
                tc,
                (const_pool, hpool, mv_pool, mg_pool, opool, psum_pool, warm_pool),
                mode,
                K,
                n_phys,
                hs,
                consts,
                out,
                chunk_tiles,
            )
    nc.finalize()
    return nc


def _order_group(counts, tiles, lo, hi):
    """Order `tiles` so prefix drift (run - 512*k) stays in [lo, hi] at
    every interior step and <= hi at the end.  DFS, largest-first."""
    tiles = sorted(tiles, key=lambda g: -counts[g])
    n = len(tiles)
    used = [False] * n
    seq = []

    def dfs(k, run):
        if k == n:
            return True
        prev = None
        for i in range(n):
            if used[i]:
                continue
            c = int(counts[tiles[i]])
            if c == prev:
                continue  # identical count -> identical subtree
            prev = c
            d = run + c - 512 * (k + 1)
            if d > hi:
                continue
            if k + 1 < n and d < lo:
                continue
            used[i] = True
            seq.append(tiles[i])
            if dfs(k + 1, run + c):
                return True
            used[i] = False
            seq.pop()
        return False

    return list(seq) if dfs(0, 0) else None


def _partition_tiles(counts, lo, hi):
    """Partition the 64 vocab tiles into 8 groups of 8, each ordered so
    packed prefix drift stays in [lo, hi].  Returns list of per-core
    sequences of global tile ids, or None."""
    rng = np.random.RandomState(0)
    base = np.argsort(counts)[::-1]
    for attempt in range(40):
        if attempt == 0:
            order = base
        else:
            order = rng.permutation(NVT)
            order = order[np.argsort(counts[order])[::-1]]
        groups = [[] for _ in range(NCORES)]
        for i, g in enumerate(order):
            rnd, pos = divmod(i, NCORES)
            c = pos if rnd % 2 == 0 else NCORES - 1 - pos
            groups[c].append(int(g))
        seqs = []
        for c in range(NCORES):
            seq = _order_group(counts, groups[c], lo, hi)
            if seq is None:
                break
            seqs.append(seq)
        if len(seqs) == NCORES:
            return seqs
    return None


def _iota_np():
    return np.tile(np.arange(P, dtype=np.float32), (P, 1))


def _pack_consts(xr, iota_np):
    return np.hstack([xr, iota_np]).astype(np.float16)


def _tilemajor(block_f16, ntiles):
    """[ntiles*P, D] fp16 -> [P, ntiles*D] tile-major."""
    return (
        block_f16.reshape(ntiles, P, D).transpose(1, 0, 2).reshape(P, ntiles * D)
    )


def _shard_mode_b(H, order, Xs, starts, groups, mode, K):
    in_maps = []
    scatter = []
    iota_np = _iota_np()
    for c in range(NCORES):
        seq = groups[c]
        rows = np.concatenate([order[starts[g] : starts[g + 1]] for g in seq])
        xval = np.concatenate([Xs[starts[g] : starts[g + 1]] for g in seq]).astype(
            np.float64
        )
        n_c = len(rows)
        block = np.zeros((NTILES_B * P, D), dtype=np.float16)
        block[:n_c] = H[rows].astype(np.float16)
        xpad = np.full(NTILES_B * P, -1000.0, dtype=np.float64)
        xpad[:n_c] = xval
        hs = _tilemajor(block, NTILES_B)
        xr = np.full((P, VT_PER_CORE * K), -1000.0, dtype=np.float32)
        for vt in range(VT_PER_CORE):
            base = 128.0 * seq[vt]
            for k in range(K):
                t = _win_lo(mode, K, vt) + k
                xr[:, vt * K + k] = (xpad[t * P : (t + 1) * P] - base).astype(
                    np.float32
                )
        in_maps.append({"hs": hs, "consts": _pack_consts(xr, iota_np)})
        scatter.append(seq)
    return in_maps, scatter


def _shard_mode_a(H, order, Xs, starts, K):
    in_maps = []
    scatter = []
    iota_np = _iota_np()
    for c in range(NCORES):
        hs = np.zeros((P, VT_PER_CORE * K * D), dtype=np.float16)
        xr = np.full((P, VT_PER_CORE * K), -1000.0, dtype=np.float32)
        seq = list(range(c * VT_PER_CORE, (c + 1) * VT_PER_CORE))
        for vt, g in enumerate(seq):
            s, e = int(starts[g]), int(starts[g + 1])
            cnt = e - s
            block = np.zeros((K * P, D), dtype=np.float16)
            block[:cnt] = H[order[s:e]].astype(np.float16)
            hs[:, vt * K * D : (vt + 1) * K * D] = _tilemajor(block, K)
            xv = np.full(K * P, -1000.0, dtype=np.float32)
            xv[:cnt] = (Xs[s:e] - g * P).astype(np.float32)
            xr[:, vt * K : (vt + 1) * K] = xv.reshape(K, P).T
        in_maps.append({"hs": hs, "consts": _pack_consts(xr, iota_np)})
        scatter.append(seq)
    return in_maps, scatter


def kernel(H, X_neis, V=V):
    global LAST_EXEC_NS, LAST_RESULTS
    H = np.asarray(H, dtype=np.float32)
    X = np.asarray(X_neis).astype(np.int64)
    assert H.shape == (N, D) and X.shape == (N,)

    order = np.argsort(X, kind="stable")
    Xs = X[order]
    counts = np.bincount(X, minlength=V).reshape(NVT, P).sum(axis=1)
    starts = np.zeros(NVT + 1, dtype=np.int64)
    np.cumsum(counts, out=starts[1:])

    groups = _partition_tiles(counts, 0, 128)
    if groups is not None:
        mode, K = "B5", 5
    else:
        groups = _partition_tiles(counts, -128, 128)
        if groups is not None:
            mode, K = "B6", 6
    if groups is not None:
        in_maps, scatter = _shard_mode_b(H, order, Xs, starts, groups, mode, K)
    else:
        mode, K = "A", max(1, int(-(-counts.max() // P)))
        in_maps, scatter = _shard_mode_a(H, order, Xs, starts, K)

    key = (mode, K)
    if key not in _PROGRAM_CACHE:
        _PROGRAM_CACHE[key] = _build_program(mode, K)
    nc = _PROGRAM_CACHE[key]

    try:
        res = run_bass_kernel_spmd(nc, in_maps, list(range(NCORES)), trace=TRACE)
    except Exception:
        # transient NRT/device hiccups have been observed; retry once
        res = run_bass_kernel_spmd(nc, in_maps, list(range(NCORES)), trace=TRACE)
    LAST_EXEC_NS = res.exec_time_ns
    LAST_RESULTS = res

    full = np.empty((V, D), dtype=np.float32)
    for c in range(NCORES):
        o = np.asarray(res.results[c]["out"], dtype=np.float32)
        for vt, g in enumerate(scatter[c]):
            full[g * P : (g + 1) * P] = o[vt * P : (vt + 1) * P]
    return full


# revision 14
# speedup vs baseline: 1.3432x; 1.0033x over previous
"""Segment-sum (AggrSum) kernel for 8 Trainium2 NeuronCores.

Math: out[v, :] = sum_{n: X_neis[n] == v} H[n, :]   (H [N, D], out [V, D])

Strategy (V-sharding with host-side bucketing as the sharding step):
  - Sort edge ids by target vocab index; group edges by 128-row vocab tile.
  - Partition the 64 vocab tiles into 8 balanced groups of 8 (one per
    core), ordered inside each group so that packed prefix drift stays
    within [0, 128] rows of 512*vt ("mode B5"). Each core reads an
    exactly-packed edge stream; every vocab tile's edges are covered by
    a fixed window of K=5 physical 128-row tiles at static offsets, and
    the one-hot masks zero out foreign rows.
  - H rows are uploaded as a single fp16 plane (rel err ~3e-4, well
    under the 2e-2 gate); one fp16 matmul per (vt, k) window tile
    accumulates into a [128, 256] fp32 PSUM tile.  Mask emission
    (one DVE is_equal per vocab tile) is split across the Vector and
    GpSimd engines so neither gates the DMA-bound stream.  PSUM->SBUF
    copies run on the Scalar (ACT) engine with an fp32->fp16 convert;
    outputs stream back as fp16 and the host upconverts + scatters.
  - Fallbacks: drift in [-128,128] with K=6 windows ("mode B6"), then
    padded per-vt tiles ("mode A") for pathological inputs.
"""

import numpy as np

import concourse.bacc as bacc
import concourse.mybir as mybir
import concourse.tile as tile
from concourse.bass_utils import run_bass_kernel_spmd

N, D, V = 32768, 256, 8192
NCORES = 8
P = 128
VT_PER_CORE = V // P // NCORES  # 8 vocab tiles of 128 per core
NVT = V // P  # 64 global vocab tiles
NTILES_B = 33  # physical 128-row tiles per core in mode B (K=5 and K=6)
N_WARMUP = 24

TRACE = False
LAST_EXEC_NS = None
LAST_RESULTS = None

_PROGRAM_CACHE: dict = {}

def _win_lo(mode: str, K: int, vt: int) -> int:
    """First physical tile of vocab tile vt's window."""
    if mode == "B5":
        return 4 * vt
    if mode == "B6":
        return 0 if vt == 0 else 4 * vt - 1
    return vt * K  # mode A: padded, disjoint windows


def _build_common(nc, tc, pools, mode, K, n_phys_tiles, hs, out, chunk_tiles):
    f32 = mybir.dt.float32
    f16 = mybir.dt.float16
    hpool, mv_pool, opool, psum_pool, warm_pool = pools
    nconst = VT_PER_CORE * K + P
    iota_off = VT_PER_CORE * K

    # Warm up the PE's HAM clock gate (throttled 1.2 GHz until ~3.4us of
    # sustained matmul activity).  Dummies run on a memset scratch tile with
    # no DMA dependency, so they start right after the engine preamble while
    # the real operands are still streaming in.
    warm_sb = warm_pool.tile([P, P], f16, name="warm_sb", tag="warmsb")
    nc.gpsimd.memset(warm_sb[:], 0.0)
    warm_ps = psum_pool.tile([P, 64], f32, name="warm", tag="warm", bufs=1)
    for _ in range(N_WARMUP):
        nc.tensor.matmul(
            out=warm_ps[:],
            lhsT=warm_sb[:],
            rhs=warm_sb[:, :64],
            start=True,
            stop=True,
        )

    # Input stream on Sync only (HWDGE SP ring) — mixing output writes into
    # the same ring halves its read throughput.  The mask operands (consts)
    # ride in front of the first chunk so one DMA + one semaphore covers
    # both and the stream starts as early as possible.
    chunks = []
    t0 = 0
    first = True
    const_sb = None
    for ct in chunk_tiles:
        if first:
            ch = hpool.tile([P, nconst + ct * D], f16, name="ch0")
            nc.sync.dma_start(ch[:], hs[:, : nconst + ct * D])
            const_sb = ch[:, :nconst]
            chunks.append((t0, ct, ch, nconst))
            first = False
        else:
            ch = hpool.tile([P, ct * D], f16, name="ch")
            nc.sync.dma_start(
                ch[:], hs[:, nconst + t0 * D : nconst + (t0 + ct) * D]
            )
            chunks.append((t0, ct, ch, 0))
        t0 += ct
    assert t0 == n_phys_tiles

    def rhs_slice(t):
        for c0, ct, ch, off0 in chunks:
            if c0 <= t < c0 + ct:
                off = off0 + (t - c0) * D
                return ch[:, off : off + D]
        raise AssertionError(t)

    # Mask emission on Vector (TRN2's Pool slot rejects TENSOR_TENSOR).
    # One slice-written buffer; subtile deps keep the matmuls fine-grained.
    big_m = mv_pool.tile([P, VT_PER_CORE * K * P], f16, name="big_m", bufs=1)

    def emit_mask(vt, k0, k1):
        # masks k0..k1-1 for vocab tile vt in one DVE op:
        # m[p, k, q] = (xrel[p, vt*K+k] == iota[q])
        nk = k1 - k0
        m = big_m[:, (vt * K + k0) * P : (vt * K + k1) * P]
        nc.vector.tensor_tensor(
            out=m.rearrange("p (k q) -> p k q", k=nk),
            in0=const_sb[:, vt * K + k0 : vt * K + k1]
            .unsqueeze(2)
            .broadcast_to([P, nk, P]),
            in1=const_sb[:, iota_off : iota_off + P]
            .unsqueeze(1)
            .broadcast_to([P, nk, P]),
            op=mybir.AluOpType.is_equal,
        )

    # First mask column alone so the PE can start the moment chunk 0 lands;
    # then the rest of vt 0, then one op per remaining vt.
    emit_mask(0, 0, 1)
    emit_mask(0, 1, K)
    for vt in range(1, VT_PER_CORE):
        emit_mask(vt, 0, K)

    big_ot = opool.tile([P, VT_PER_CORE * D], f16, name="big_ot", bufs=1)

    # Output: psum->sbuf fp16 copies on Scalar (ACT), grouped write-back
    # DMAs on Scalar's HWDGE ring (keeps the Sync/input ring read-only).
    OUT_GROUPS = [(0, 3), (3, 6), (6, 8)]
    for vt in range(VT_PER_CORE):
        ps = psum_pool.tile([P, D], f32, name="ps")
        for k in range(K):
            t = _win_lo(mode, K, vt) + k
            nc.tensor.matmul(
                out=ps[:],
                lhsT=big_m[:, (vt * K + k) * P : (vt * K + k + 1) * P],
                rhs=rhs_slice(t),
                start=(k == 0),
                stop=(k == K - 1),
            )
        ot = big_ot[:, vt * D : (vt + 1) * D]
        nc.scalar.copy(ot, ps[:])
        for g0, g1 in OUT_GROUPS:
            if vt == g1 - 1:
                nc.scalar.dma_start(
                    out[:, g0 * D : g1 * D], big_ot[:, g0 * D : g1 * D]
                )


def _build_program(mode, K):
    """mode 'B5'/'B6': exact-packed windows; mode 'A': padded (K tiles/vt)."""
    f16 = mybir.dt.float16
    if mode in ("B5", "B6"):
        n_phys = NTILES_B
        chunk_tiles = [1, 6, 8, 9, 9]
    else:
        n_phys = VT_PER_CORE * K
        nt = n_phys
        chunk_tiles = []
        while nt > 0:
            chunk_tiles.append(min(7, nt))
            nt -= min(7, nt)
    nconst = VT_PER_CORE * K + P

    nc = bacc.Bacc("TRN2", target_bir_lowering=False)
    hs = nc.dram_tensor("hs", [P, nconst + n_phys * D], f16, kind="ExternalInput")
    out = nc.dram_tensor("out", [P, VT_PER_CORE * D], f16, kind="ExternalOutput")

    with tile.TileContext(nc) as tc:
        with (
            tc.tile_pool(name="h", bufs=min(len(chunk_tiles), 16)) as hpool,
            tc.tile_pool(name="mv", bufs=1) as mv_pool,
            tc.tile_pool(name="o", bufs=1) as opool,
            tc.tile_pool(name="warm", bufs=1) as warm_pool,
            tc.tile_pool(name="psum", bufs=4, space="PSUM") as psum_pool,
        ):
            _build_common(
                nc,
                tc,
                (hpool, mv_pool, opool, psum_pool, warm_pool),
                mode,
                K,
                n_phys,
                hs,
                out,
                chunk_tiles,
            )
    nc.finalize()
    return nc


def _order_group(counts, tiles, lo, hi):
    """Order `tiles` so prefix drift (run - 512*k) stays in [lo, hi] at
    every interior step and <= hi at the end.  DFS, largest-first."""
    tiles = sorted(tiles, key=lambda g: -counts[g])
    n = len(tiles)
    used = [False] * n
    seq = []

    def dfs(k, run):
        if k == n:
            return True
        prev = None
        for i in range(n):
            if used[i]:
                continue
            c = int(counts[tiles[i]])
            if c == prev:
                continue  # identical count -> identical subtree
            prev = c
            d = run + c - 512 * (k + 1)
            if d > hi:
                continue
            if k + 1 < n and d < lo:
                continue
            used[i] = True
            seq.append(tiles[i])
            if dfs(k + 1, run + c):
                return True
            used[i] = False
            seq.pop()
        return False

    return list(seq) if dfs(0, 0) else None


def _partition_tiles(counts, lo, hi):
    """Partition the 64 vocab tiles into 8 groups of 8, each ordered so
    packed prefix drift stays in [lo, hi].  Returns list of per-core
    sequences of global tile ids, or None."""
    rng = np.random.RandomState(0)
    base = np.argsort(counts)[::-1]
    for attempt in range(40):
        if attempt == 0:
            order = base
        else:
            order = rng.permutation(NVT)
            order = order[np.argsort(counts[order])[::-1]]
        groups = [[] for _ in range(NCORES)]
        for i, g in enumerate(order):
            rnd, pos = divmod(i, NCORES)
            c = pos if rnd % 2 == 0 else NCORES - 1 - pos
            groups[c].append(int(g))
        seqs = []
        for c in range(NCORES):
            seq = _order_group(counts, groups[c], lo, hi)
            if seq is None:
                break
            seqs.append(seq)
        if len(seqs) == NCORES:
            return seqs
    return None


def _iota_np():
    return np.tile(np.arange(P, dtype=np.float32), (P, 1))


def _pack_hs(xr, iota_np, hs_tiles):
    """consts ([P, nk] xrel + [P, 128] iota) prepended to the tile-major
    H stream -> single [P, nconst + ntiles*D] fp16 input."""
    return np.hstack([xr, iota_np, hs_tiles]).astype(np.float16)


def _tilemajor(block_f16, ntiles):
    """[ntiles*P, D] fp16 -> [P, ntiles*D] tile-major."""
    return (
        block_f16.reshape(ntiles, P, D).transpose(1, 0, 2).reshape(P, ntiles * D)
    )


def _shard_mode_b(H, order, Xs, starts, groups, mode, K):
    in_maps = []
    scatter = []
    iota_np = _iota_np()
    for c in range(NCORES):
        seq = groups[c]
        rows = np.concatenate([order[starts[g] : starts[g + 1]] for g in seq])
        xval = np.concatenate([Xs[starts[g] : starts[g + 1]] for g in seq]).astype(
            np.float64
        )
        n_c = len(rows)
        block = np.zeros((NTILES_B * P, D), dtype=np.float16)
        block[:n_c] = H[rows].astype(np.float16)
        xpad = np.full(NTILES_B * P, -1000.0, dtype=np.float64)
        xpad[:n_c] = xval
        hs_tiles = _tilemajor(block, NTILES_B)
        xr = np.full((P, VT_PER_CORE * K), -1000.0, dtype=np.float32)
        for vt in range(VT_PER_CORE):
            base = 128.0 * seq[vt]
            for k in range(K):
                t = _win_lo(mode, K, vt) + k
                xr[:, vt * K + k] = (xpad[t * P : (t + 1) * P] - base).astype(
                    np.float32
                )
        in_maps.append({"hs": _pack_hs(xr, iota_np, hs_tiles)})
        scatter.append(seq)
    return in_maps, scatter


def _shard_mode_a(H, order, Xs, starts, K):
    in_maps = []
    scatter = []
    iota_np = _iota_np()
    for c in range(NCORES):
        hs = np.zeros((P, VT_PER_CORE * K * D), dtype=np.float16)
        xr = np.full((P, VT_PER_CORE * K), -1000.0, dtype=np.float32)
        seq = list(range(c * VT_PER_CORE, (c + 1) * VT_PER_CORE))
        for vt, g in enumerate(seq):
            s, e = int(starts[g]), int(starts[g + 1])
            cnt = e - s
            block = np.zeros((K * P, D), dtype=np.float16)
            block[:cnt] = H[order[s:e]].astype(np.float16)
            hs[:, vt * K * D : (vt + 1) * K * D] = _tilemajor(block, K)
            xv = np.full(K * P, -1000.0, dtype=np.float32)
            xv[:cnt] = (Xs[s:e] - g * P).astype(np.float32)
            xr[:, vt * K : (vt + 1) * K] = xv.reshape(K, P).T
        in_maps.append({"hs": _pack_hs(xr, iota_np, hs)})
        scatter.append(seq)
    return in_maps, scatter


def kernel(H, X_neis, V=V):
    global LAST_EXEC_NS, LAST_RESULTS
    H = np.asarray(H, dtype=np.float32)
    X = np.asarray(X_neis).astype(np.int64)
    assert H.shape == (N, D) and X.shape == (N,)

    order = np.argsort(X, kind="stable")
    Xs = X[order]
    counts = np.bincount(X, minlength=V).reshape(NVT, P).sum(axis=1)
    starts = np.zeros(NVT + 1, dtype=np.int64)
    np.cumsum(counts, out=starts[1:])

    groups = _partition_tiles(counts, 0, 128)
    if groups is not None:
        mode, K = "B5", 5
    else:
        groups = _partition_tiles(counts, -128, 128)
        if groups is not None:
            mode, K = "B6", 6
    if groups is not None:
        in_maps, scatter = _shard_mode_b(H, order, Xs, starts, groups, mode, K)
    else:
        mode, K = "A", max(1, int(-(-counts.max() // P)))
        in_maps, scatter = _shard_mode_a(H, order, Xs, starts, K)

    key = (mode, K)
    if key not in _PROGRAM_CACHE:
        _PROGRAM_CACHE[key] = _build_program(mode, K)
    nc = _PROGRAM_CACHE[key]

    try:
        res = run_bass_kernel_spmd(nc, in_maps, list(range(NCORES)), trace=TRACE)
    except Exception:
        # transient NRT/device hiccups have been observed; retry once
        res = run_bass_kernel_spmd(nc, in_maps, list(range(NCORES)), trace=TRACE)
    LAST_EXEC_NS = res.exec_time_ns
    LAST_RESULTS = res

    full = np.empty((V, D), dtype=np.float32)
    for c in range(NCORES):
        o = np.asarray(res.results[c]["out"], dtype=np.float32)  # [P, VT*D]
        for vt, g in enumerate(scatter[c]):
            full[g * P : (g + 1) * P] = o[:, vt * D : (vt + 1) * D]
    return full
